# revision 6
# baseline (speedup 1.0000x reference)
"""Trainium2 Bass kernel for nn_Metamorph_parameterReinforcer.

Math background (exact identities, verified against the reference):
  The reference's einsum("bfp,mn->bfm", fx, wfft) sums over BOTH p and n,
  so each "STFT block" collapses:
    sum_p fft(x, norm=forward)[..., p] == x[..., 0]
    block(x)[b, f, k] = Re tanh(x[b, f, 0] * W[k]),
       W[k] = sum_m (sum_n wfft[m, n]) * exp(2j*pi*k*m/64)
  Chaining three blocks, only element 0 of the last axis propagates:
    a  = params[:, :, 0]
    s1 = Retanh(a  * W0[0]);  s2 = Retanh(s1 * W1[0])
    x3[b, f, l] = Retanh(s2[b, f] * W2[l])         # (512, 1000, 64)
    h  = tanh(x3.reshape(512, 64000) @ lin1_w.T + lin1_b)
    out = sigmoid(h @ lin2_w.T + lin2_b)
  Because |W0[0]|, |W1[0]| ~ 32000 (sums of 64000 uniforms), tanh saturates
  and s2 is exactly +-1 in f32 for all but (rare) |a| < ~1e-4 entries. Where
  s2 is exactly +-1, x3[b, f, :] = s2[b, f] * X1[:] with X1 = Retanh(W2) --
  exactly rank-1. So
    H_pre[b, j] = sum_f s2[b, f] * A[j, f] + lin1_b[j] (+ rare corrections)
    A[j, f]     = sum_l X1[l] * w1[j, 64 f + l]
  A is a small (1000 x 1000) fold of the lin1 weight against the spectral
  vector X1; it is precomputed on host alongside s1/s2/X1 (the same style of
  closed-form host collapse the spectral stages already use).  The lin1 bias,
  the rare non-saturated (b, f) entries, AND the fp8 quantization residual of
  s2 are all folded EXACTLY into the device contraction as extra K rows
  (rank-1 terms: one-hot batch indicator x f32 correction row).

Device kernel (8 cores as a 4x2 grid: 4-way shard of lin1 output dim j
(250 each), 2-way shard of batch b (256 each)):
  stage 2: H[j, b] = tanh(sum_k at[k, j] * s2q[k, b])   (TensorE + ScalarE)
           at bf16 (lhsT), s2q fp8e4 (rhs, +-1 a.e. -- exact); k ranges over
           f=0..999, a bias row, then the rank-1 fix rows.
  stage 3: partial[o, b] = sum_j l2t[j, o] * H[j, b]    (TensorE)
Host combines the 4 j-shard partials per b-shard: out = sigmoid(... + lin2_b).
Inputs stream as two tensors (bf16 wb = l2t block + at tiles on the sync
HWDGE ring; fp8 sb = s2q tiles on the scalar ring), chunked so the matmuls
chase the DMAs.  A few warm-up matmuls on a memset tile run during the DMA
lead-in to flip the PE HAM clock gate to 2.4 GHz before the real stream.
The tail is split by batch half (tanh halves, 4 stage-3 matmuls, two output
DMAs on the two rings) so most of it overlaps the last stage-2 work.
"""

import numpy as np

B, MODES, L = 512, 1000, 64
NCORES = 8
JG, BG = 4, 2                  # core grid: j-shards x b-shards
JSH = MODES // JG              # 250 lin1 output rows per core
BSH = B // BG                  # 256 batch columns per core
BH = BSH // 2                  # 128 batch cols per output half
JH = JSH // 2                  # 125 = matmul M (PSUM partition limit 128)
L2C = 128                      # l2t block: 2 halves of [125, 64]
NJUNK = 5                      # PE warm-up matmuls (N=512 each)
SAT = 50.0                     # |2*s*Re(W)| beyond this: Retanh == sign


def _retanh(s, w):
    """Re tanh(s * w) for real array s and complex (array or scalar) w."""
    s = np.asarray(s, np.float64)
    x = 2.0 * np.multiply.outer(s, np.real(w))
    y = 2.0 * np.multiply.outer(s, np.imag(w))
    xc = np.clip(x, -SAT, SAT)
    with np.errstate(over="ignore", invalid="ignore"):
        r = np.sinh(xc) / (np.cosh(xc) + np.cos(y))
    return np.where(np.abs(x) >= SAT, np.sign(x), r)


def _wvec(wre, wim):
    """W[k] = sum_m (sum_n w[m, n]) * exp(2j pi k m / L)."""
    wsum = wre.astype(np.float64).sum(axis=1) + 1j * wim.astype(np.float64).sum(axis=1)
    tw = np.exp(2j * np.pi * np.outer(np.arange(L), np.arange(L)) / L)
    return tw @ wsum


_CACHE = {}


def _chunk_groups(n_kt, first, rest):
    """Split tiles 0..n_kt-1 into chunks: first `first` tiles, then `rest`-sized."""
    groups = [list(range(min(first, n_kt)))]
    done = min(first, n_kt)
    while done < n_kt:
        take = min(rest, n_kt - done)
        groups.append(list(range(done, done + take)))
        done += take
    return groups


def _build_program(n_kt):
    """Build (and cache) the Bass program. Same program for all 8 cores."""
    key = ("prog", "v3", n_kt)
    if key in _CACHE:
        return _CACHE[key]

    import concourse.bacc as bacc
    import concourse.mybir as mybir
    import concourse.tile as tile

    f32 = mybir.dt.float32
    bf16 = mybir.dt.bfloat16
    fp8 = mybir.dt.float8e4
    nc = bacc.Bacc("TRN2", target_bir_lowering=False, debug=False)

    nbw = L2C + JSH * n_kt
    nbs = BSH * n_kt
    wb_d = nc.dram_tensor("wb", [128, nbw], bf16, kind="ExternalInput")
    sb_d = nc.dram_tensor("sb", [128, nbs], fp8, kind="ExternalInput")
    outp_d = nc.dram_tensor("outp", [L, BSH], f32, kind="ExternalOutput")

    with tile.TileContext(nc) as tc:
        with (
            tc.tile_pool(name="const", bufs=1) as const,
            tc.tile_pool(name="acc", bufs=1) as acc,
            tc.tile_pool(name="psJ", bufs=1, space="PSUM") as psJ,
            tc.tile_pool(name="psH", bufs=1, space="PSUM") as psH,
            tc.tile_pool(name="psO", bufs=1, space="PSUM") as psO,
        ):
            # ---- PE warm-up: flip the HAM clock gate during the DMA lead-in
            junk = const.tile([128, 512], bf16)
            nc.vector.memset(junk[:, :], 1.0)
            jp = psJ.tile([128, 512], f32)
            for _ in range(NJUNK):
                nc.tensor.matmul(
                    jp[:, :], junk[:, 0:128], junk[:, :], start=True, stop=True
                )

            # ---- input streaming: wb chunks on sync ring, sb on scalar ring
            wb = const.tile([128, nbw], bf16)
            sb = const.tile([128, nbs], fp8)
            wc = _chunk_groups(n_kt, 1, 3)
            c0 = 0
            for g in wc:
                c1 = L2C + JSH * (g[-1] + 1)
                nc.sync.dma_start(wb[:, c0:c1], wb_d.ap()[:, c0:c1])
                c0 = c1
            sc = _chunk_groups(n_kt, 3, 3)
            c0 = 0
            for g in sc:
                c1 = BSH * (g[-1] + 1)
                nc.scalar.dma_start(sb[:, c0:c1], sb_d.ap()[:, c0:c1])
                c0 = c1

            # ---- stage 2: H[j, b] = tanh(sum_k at[k, j] s2q[k, b]) ----
            ph0 = psH.tile([JH, BSH], f32)
            ph1 = psH.tile([JH, BSH], f32)
            hs = []
            for ph_i, ph in enumerate((ph0, ph1)):
                j0 = L2C + JH * ph_i
                for t in range(n_kt):
                    nc.tensor.matmul(
                        ph[:, :],
                        wb[0:128, j0 + JSH * t : j0 + JSH * t + JH],
                        sb[0:128, BSH * t : BSH * (t + 1)],
                        start=(t == 0),
                        stop=(t == n_kt - 1),
                    )
                h = acc.tile([JH, BSH], bf16)
                if ph_i == 0:
                    # overlaps the ph1 matmuls on the ACT engine
                    nc.scalar.activation(
                        h[:, :], ph[:, :], mybir.ActivationFunctionType.Tanh
                    )
                else:
                    # split so the stage-3 A-half can start sooner
                    nc.scalar.activation(
                        h[:, 0:BH], ph[:, 0:BH], mybir.ActivationFunctionType.Tanh
                    )
                    nc.scalar.activation(
                        h[:, BH:BSH], ph[:, BH:BSH], mybir.ActivationFunctionType.Tanh
                    )
                hs.append(h)

            # ---- stage 3 (by batch half): partial[o, b] = sum_j l2t[j, o] h[j, b]
            po0 = psO.tile([L, BH], f32)
            po1 = psO.tile([L, BH], f32)
            po = [po0, po1]
            for bh in range(2):            # first matmul of each half needs only h0
                nc.tensor.matmul(
                    po[bh][:, :],
                    wb[0:JH, 0:64],
                    hs[0][:, BH * bh : BH * (bh + 1)],
                    start=True,
                    stop=False,
                )
            o_sb = acc.tile([L, BSH], f32)
            out_eng = [nc.sync, nc.scalar]
            for bh in range(2):
                nc.tensor.matmul(
                    po[bh][:, :],
                    wb[0:JH, 64:128],
                    hs[1][:, BH * bh : BH * (bh + 1)],
                    start=False,
                    stop=True,
                )
                nc.vector.tensor_copy(o_sb[:, BH * bh : BH * (bh + 1)], po[bh][:, :])
                out_eng[bh].dma_start(
                    outp_d.ap()[:, BH * bh : BH * (bh + 1)],
                    o_sb[:, BH * bh : BH * (bh + 1)],
                )

    nc.compile()
    _CACHE[key] = nc
    return nc


def profile_last(trace_cores=None):
    """Re-run the last-built program with NTFF tracing (dev/test helper)."""
    if "last_run" not in _CACHE:
        return None
    from concourse.bass_utils import run_bass_kernel_spmd

    nc, in_maps = _CACHE["last_run"]
    return run_bass_kernel_spmd(
        nc,
        in_maps,
        list(range(NCORES)),
        trace=True,
        trace_cores=trace_cores,
    )


def _host_prep(
    params, wfft0_re, wfft0_im, wfft1_re, wfft1_im, wfft2_re, wfft2_im,
    lin1_w, lin1_b, lin2_w,
):
    """Closed-form spectral collapse + per-core device input construction."""
    import ml_dtypes

    bf16 = ml_dtypes.bfloat16

    a = params[:, :, 0].astype(np.float64)
    w0 = _wvec(wfft0_re, wfft0_im)[0]
    w1v = _wvec(wfft1_re, wfft1_im)[0]
    w2 = _wvec(wfft2_re, wfft2_im)
    s1 = _retanh(a, w0)
    s2 = _retanh(s1, w1v).astype(np.float32)          # (512, 1000), +-1 a.e.
    x1 = _retanh(np.float64(1.0), w2).astype(np.float32)   # (64,)

    # fold of lin1_w against the spectral vector: A[j,f] = sum_l X1[l] w1[j,64f+l]
    A = (lin1_w.reshape(-1, L) @ x1).reshape(MODES, MODES)  # (j, f) f32

    import concourse.mybir as mybir

    np_fp8 = mybir.dt.np(mybir.dt.float8e4)
    s2q = s2.astype(np_fp8)                           # exact on +-1 entries
    s2qf = s2q.astype(np.float64)

    # rank-1 corrections: non-saturated tanh entries + s2 fp8 residual
    bad_b, bad_f = np.nonzero(np.abs(s2) != np.float32(1.0))
    x1_64 = x1.astype(np.float64)
    A64 = A.astype(np.float64)
    vrows = {}                                        # b -> correction row (1000,)
    for b, f in zip(bad_b.tolist(), bad_f.tolist()):
        s = np.float64(s2[b, f])
        delta = _retanh(s, w2) - s * x1_64            # (64,) x3 vs rank-1 fix
        row = vrows.get(b)
        if row is None:
            row = np.zeros(MODES, np.float64)
            vrows[b] = row
        row += lin1_w[:, 64 * f : 64 * (f + 1)].astype(np.float64) @ delta
        dq = np.float64(s2[b, f]) - s2qf[b, f]        # fp8 residual fix
        if dq != 0.0:
            row += dq * A64[:, f]

    ext = [[b for b in sorted(vrows) if b // BSH == bg] for bg in range(BG)]
    n_k = MODES + 1 + max(len(e) for e in ext)        # f rows + bias row + fixes
    n_kt = (n_k + 127) // 128
    nbw = L2C + JSH * n_kt
    nbs = BSH * n_kt

    atT = np.ascontiguousarray(A.T)                   # (f, j)
    s2qT = np.ascontiguousarray(s2q.T)                # (f, b) fp8

    in_maps = []
    for c in range(NCORES):
        jg, bg = c // BG, c % BG
        j0, b0 = JSH * jg, BSH * bg
        wbm = np.zeros((128, nbw), np.float32)
        sbm = np.zeros((128, nbs), np_fp8)
        for hh in range(2):                           # l2t block
            jlo = j0 + JH * hh
            wbm[0:JH, 64 * hh : 64 * hh + 64] = lin2_w[:, jlo : jlo + JH].T
        for t in range(n_kt):
            k0 = 128 * t
            frows = max(0, min(128, MODES - k0))
            if frows > 0:
                wbm[0:frows, L2C + JSH * t : L2C + JSH * t + JSH] = atT[
                    k0 : k0 + frows, j0 : j0 + JSH
                ]
                sbm[0:frows, BSH * t : BSH * t + BSH] = s2qT[
                    k0 : k0 + frows, b0 : b0 + BSH
                ]
            for r in range(frows, 128):
                k = k0 + r
                if k == MODES:                        # bias row
                    wbm[r, L2C + JSH * t : L2C + JSH * t + JSH] = lin1_b[j0 : j0 + JSH]
                    sbm[r, BSH * t : BSH * t + BSH] = np_fp8(1.0)
                elif MODES < k < MODES + 1 + len(ext[bg]):
                    be = ext[bg][k - MODES - 1]
                    wbm[r, L2C + JSH * t : L2C + JSH * t + JSH] = vrows[be][
                        j0 : j0 + JSH
                    ]
                    sbm[r, BSH * t + (be - b0)] = np_fp8(1.0)
        in_maps.append(
            {
                "wb": np.ascontiguousarray(wbm.astype(bf16)),
                "sb": np.ascontiguousarray(sbm),
            }
        )
    return in_maps, n_kt


def kernel(
    params,
    wfft0_re,
    wfft0_im,
    wfft1_re,
    wfft1_im,
    wfft2_re,
    wfft2_im,
    lin1_w,
    lin1_b,
    lin2_w,
    lin2_b,
):
    from concourse.bass_utils import run_bass_kernel_spmd

    in_maps, n_kt = _host_prep(
        params, wfft0_re, wfft0_im, wfft1_re, wfft1_im, wfft2_re, wfft2_im,
        lin1_w, lin1_b, lin2_w,
    )

    nc = _build_program(n_kt)
    _CACHE["last_run"] = (nc, in_maps)
    res = run_bass_kernel_spmd(nc, in_maps, list(range(NCORES)))

    # host: sum j-shard partials per b-shard, add lin2 bias, sigmoid
    pre = np.zeros((L, B), np.float64)
    for c in range(NCORES):
        bg = c % BG
        pre[:, BSH * bg : BSH * (bg + 1)] += res.results[c]["outp"].astype(np.float64)
    out = 1.0 / (1.0 + np.exp(-(pre.T + lin2_b.astype(np.float64))))
    return out.astype(np.float32)


# revision 7
# speedup vs baseline: 1.0224x; 1.0224x over previous
"""Trainium2 Bass kernel for nn_Metamorph_parameterReinforcer.

Math background (exact identities, verified against the reference):
  The reference's einsum("bfp,mn->bfm", fx, wfft) sums over BOTH p and n,
  so each "STFT block" collapses:
    sum_p fft(x, norm=forward)[..., p] == x[..., 0]
    block(x)[b, f, k] = Re tanh(x[b, f, 0] * W[k]),
       W[k] = sum_m (sum_n wfft[m, n]) * exp(2j*pi*k*m/64)
  Chaining three blocks, only element 0 of the last axis propagates:
    a  = params[:, :, 0]
    s1 = Retanh(a  * W0[0]);  s2 = Retanh(s1 * W1[0])
    x3[b, f, l] = Retanh(s2[b, f] * W2[l])         # (512, 1000, 64)
    h  = tanh(x3.reshape(512, 64000) @ lin1_w.T + lin1_b)
    out = sigmoid(h @ lin2_w.T + lin2_b)
  Because |W0[0]|, |W1[0]| ~ 32000 (sums of 64000 uniforms), tanh saturates
  and s2 is exactly +-1 in f32 for all but (rare) |a| < ~1e-4 entries. Where
  s2 is exactly +-1, x3[b, f, :] = s2[b, f] * X1[:] with X1 = Retanh(W2) --
  exactly rank-1. So
    H_pre[b, j] = sum_f s2[b, f] * A[j, f] + lin1_b[j] (+ rare corrections)
    A[j, f]     = sum_l X1[l] * w1[j, 64 f + l]
  A is a small (1000 x 1000) fold of the lin1 weight against the spectral
  vector X1; it is precomputed on host alongside s1/s2/X1 (the same style of
  closed-form host collapse the spectral stages already use).  The lin1 bias,
  the rare non-saturated (b, f) entries, AND the fp8 quantization residual of
  s2 are all folded EXACTLY into the device contraction as extra K rows
  (rank-1 terms: one-hot batch indicator x f32 correction row).

Device kernel (8 cores as a 4x2 grid: 4-way shard of lin1 output dim j
(250 each), 2-way shard of batch b (256 each)):
  stage 2: H[j, b] = tanh(sum_k at[k, j] * s2q[k, b])   (TensorE + ScalarE)
           at bf16 (lhsT), s2q fp8e4 (rhs, +-1 a.e. -- exact); k ranges over
           f=0..999, a bias row, then the rank-1 fix rows.
  stage 3: partial[o, b] = sum_j l2t[j, o] * H[j, b]    (TensorE)
Host combines the 4 j-shard partials per b-shard: out = sigmoid(... + lin2_b).
Inputs stream as two tensors (bf16 wb = l2t block + at tiles on the sync
HWDGE ring; fp8 sb = s2q tiles on the scalar ring), chunked so the matmuls
chase the DMAs.  A few warm-up matmuls on a memset tile run during the DMA
lead-in to flip the PE HAM clock gate to 2.4 GHz before the real stream.
The tail is split by batch half (tanh halves, 4 stage-3 matmuls, two output
DMAs on the two rings) so most of it overlaps the last stage-2 work.
"""

import numpy as np

B, MODES, L = 512, 1000, 64
NCORES = 8
JG, BG = 4, 2                  # core grid: j-shards x b-shards
JSH = MODES // JG              # 250 lin1 output rows per core
BSH = B // BG                  # 256 batch columns per core
BH = BSH // 2                  # 128 batch cols per output half
JH = JSH // 2                  # 125 = matmul M (PSUM partition limit 128)
L2C = 128                      # l2t block: 2 halves of [125, 64]
NJUNK = 5                      # PE warm-up matmuls (N=512 each)
SAT = 50.0                     # |2*s*Re(W)| beyond this: Retanh == sign


def _retanh(s, w):
    """Re tanh(s * w) for real array s and complex (array or scalar) w."""
    s = np.asarray(s, np.float64)
    x = 2.0 * np.multiply.outer(s, np.real(w))
    y = 2.0 * np.multiply.outer(s, np.imag(w))
    xc = np.clip(x, -SAT, SAT)
    with np.errstate(over="ignore", invalid="ignore"):
        r = np.sinh(xc) / (np.cosh(xc) + np.cos(y))
    return np.where(np.abs(x) >= SAT, np.sign(x), r)


def _wvec(wre, wim):
    """W[k] = sum_m (sum_n w[m, n]) * exp(2j pi k m / L)."""
    wsum = wre.astype(np.float64).sum(axis=1) + 1j * wim.astype(np.float64).sum(axis=1)
    tw = np.exp(2j * np.pi * np.outer(np.arange(L), np.arange(L)) / L)
    return tw @ wsum


_CACHE = {}


def _chunk_groups(n_kt, first, rest):
    """Split tiles 0..n_kt-1 into chunks: first `first` tiles, then `rest`-sized."""
    groups = [list(range(min(first, n_kt)))]
    done = min(first, n_kt)
    while done < n_kt:
        take = min(rest, n_kt - done)
        groups.append(list(range(done, done + take)))
        done += take
    return groups


def _build_program(n_kt):
    """Build (and cache) the Bass program. Same program for all 8 cores."""
    key = ("prog", "v3", n_kt)
    if key in _CACHE:
        return _CACHE[key]

    import concourse.bacc as bacc
    import concourse.mybir as mybir
    import concourse.tile as tile

    f32 = mybir.dt.float32
    bf16 = mybir.dt.bfloat16
    fp8 = mybir.dt.float8e4
    nc = bacc.Bacc("TRN2", target_bir_lowering=False, debug=False)

    nbw = L2C + JSH * n_kt
    nbs = BSH * n_kt
    wb_d = nc.dram_tensor("wb", [128, nbw], bf16, kind="ExternalInput")
    sb_d = nc.dram_tensor("sb", [128, nbs], fp8, kind="ExternalInput")
    outp_d = nc.dram_tensor("outp", [L, BSH], f32, kind="ExternalOutput")

    with tile.TileContext(nc) as tc:
        with (
            tc.tile_pool(name="const", bufs=1) as const,
            tc.tile_pool(name="acc", bufs=1) as acc,
            tc.tile_pool(name="psJ", bufs=1, space="PSUM") as psJ,
            tc.tile_pool(name="psH", bufs=1, space="PSUM") as psH,
            tc.tile_pool(name="psO", bufs=1, space="PSUM") as psO,
        ):
            # ---- PE warm-up: flip the HAM clock gate during the DMA lead-in
            junk = const.tile([128, 512], bf16)
            nc.vector.memset(junk[:, :], 1.0)
            jp = psJ.tile([128, 512], f32)
            for _ in range(NJUNK):
                nc.tensor.matmul(
                    jp[:, :], junk[:, 0:128], junk[:, :], start=True, stop=True
                )

            # ---- input streaming: wb chunks on sync ring, sb on scalar ring
            # (tiny first chunks so the PE starts early; tiny last chunks so
            # the final completion-receipt lag covers little data)
            wb = const.tile([128, nbw], bf16)
            sb = const.tile([128, nbs], fp8)
            wc = [[0]] + _chunk_groups(n_kt - 1, 3, 3)
            for g in wc[1:]:
                for i in range(len(g)):
                    g[i] += 1
            c0 = 0
            for g in wc:
                c1 = L2C + JSH * (g[-1] + 1)
                nc.sync.dma_start(wb[:, c0:c1], wb_d.ap()[:, c0:c1])
                c0 = c1
            c0 = 0
            for g in wc:
                c1 = BSH * (g[-1] + 1)
                nc.scalar.dma_start(sb[:, c0:c1], sb_d.ap()[:, c0:c1])
                c0 = c1

            # ---- stage 2: H[j, b] = tanh(sum_k at[k, j] s2q[k, b]) ----
            # Both halves run tiles 0..n-2 first; the two tile-(n-1) matmuls
            # go last so each PSUM group closes right after the final chunk.
            ph0 = psH.tile([JH, BSH], f32)
            ph1 = psH.tile([JH, BSH], f32)
            phs = (ph0, ph1)

            def s2mm(ph_i, t):
                j0 = L2C + JH * ph_i
                nc.tensor.matmul(
                    phs[ph_i][:, :],
                    wb[0:128, j0 + JSH * t : j0 + JSH * t + JH],
                    sb[0:128, BSH * t : BSH * (t + 1)],
                    start=(t == 0),
                    stop=(t == n_kt - 1),
                    skip_group_check=True,
                )

            for ph_i in range(2):
                for t in range(n_kt - 1):
                    s2mm(ph_i, t)
            s2mm(0, n_kt - 1)
            s2mm(1, n_kt - 1)

            h0 = acc.tile([JH, BSH], bf16)
            h1 = acc.tile([JH, BSH], bf16)
            nc.scalar.activation(
                h0[:, :], ph0[:, :], mybir.ActivationFunctionType.Tanh
            )
            nc.scalar.activation(
                h1[:, :], ph1[:, :], mybir.ActivationFunctionType.Tanh
            )

            # ---- stage 3: partial[o, b] = sum_j l2t[j, o] h[j, b] ----
            po = psO.tile([L, BSH], f32)
            nc.tensor.matmul(
                po[:, :], wb[0:JH, 0:64], h0[:, :], start=True, stop=False
            )
            nc.tensor.matmul(
                po[:, :], wb[0:JH, 64:128], h1[:, :], start=False, stop=True
            )
            o_sb = acc.tile([L, BSH], f32)
            nc.vector.tensor_copy(o_sb[:, :], po[:, :])
            # two output DMAs on the two rings: receipts overlap
            nc.sync.dma_start(outp_d.ap()[:, 0:BH], o_sb[:, 0:BH])
            nc.scalar.dma_start(outp_d.ap()[:, BH:BSH], o_sb[:, BH:BSH])

    nc.compile()
    _CACHE[key] = nc
    return nc


def profile_last(trace_cores=None):
    """Re-run the last-built program with NTFF tracing (dev/test helper)."""
    if "last_run" not in _CACHE:
        return None
    from concourse.bass_utils import run_bass_kernel_spmd

    nc, in_maps = _CACHE["last_run"]
    return run_bass_kernel_spmd(
        nc,
        in_maps,
        list(range(NCORES)),
        trace=True,
        trace_cores=trace_cores,
    )


def _host_prep(
    params, wfft0_re, wfft0_im, wfft1_re, wfft1_im, wfft2_re, wfft2_im,
    lin1_w, lin1_b, lin2_w,
):
    """Closed-form spectral collapse + per-core device input construction."""
    import ml_dtypes

    bf16 = ml_dtypes.bfloat16

    a = params[:, :, 0].astype(np.float64)
    w0 = _wvec(wfft0_re, wfft0_im)[0]
    w1v = _wvec(wfft1_re, wfft1_im)[0]
    w2 = _wvec(wfft2_re, wfft2_im)
    s1 = _retanh(a, w0)
    s2 = _retanh(s1, w1v).astype(np.float32)          # (512, 1000), +-1 a.e.
    x1 = _retanh(np.float64(1.0), w2).astype(np.float32)   # (64,)

    # fold of lin1_w against the spectral vector: A[j,f] = sum_l X1[l] w1[j,64f+l]
    A = (lin1_w.reshape(-1, L) @ x1).reshape(MODES, MODES)  # (j, f) f32

    import concourse.mybir as mybir

    np_fp8 = mybir.dt.np(mybir.dt.float8e4)
    s2q = s2.astype(np_fp8)                           # exact on +-1 entries
    s2qf = s2q.astype(np.float64)

    # rank-1 corrections: non-saturated tanh entries + s2 fp8 residual
    bad_b, bad_f = np.nonzero(np.abs(s2) != np.float32(1.0))
    x1_64 = x1.astype(np.float64)
    A64 = A.astype(np.float64)
    vrows = {}                                        # b -> correction row (1000,)
    for b, f in zip(bad_b.tolist(), bad_f.tolist()):
        s = np.float64(s2[b, f])
        delta = _retanh(s, w2) - s * x1_64            # (64,) x3 vs rank-1 fix
        row = vrows.get(b)
        if row is None:
            row = np.zeros(MODES, np.float64)
            vrows[b] = row
        row += lin1_w[:, 64 * f : 64 * (f + 1)].astype(np.float64) @ delta
        dq = np.float64(s2[b, f]) - s2qf[b, f]        # fp8 residual fix
        if dq != 0.0:
            row += dq * A64[:, f]

    ext = [[b for b in sorted(vrows) if b // BSH == bg] for bg in range(BG)]
    n_k = MODES + 1 + max(len(e) for e in ext)        # f rows + bias row + fixes
    n_kt = (n_k + 127) // 128
    nbw = L2C + JSH * n_kt
    nbs = BSH * n_kt

    atT = np.ascontiguousarray(A.T)                   # (f, j)
    s2qT = np.ascontiguousarray(s2q.T)                # (f, b) fp8

    in_maps = []
    for c in range(NCORES):
        jg, bg = c // BG, c % BG
        j0, b0 = JSH * jg, BSH * bg
        wbm = np.zeros((128, nbw), np.float32)
        sbm = np.zeros((128, nbs), np_fp8)
        for hh in range(2):                           # l2t block
            jlo = j0 + JH * hh
            wbm[0:JH, 64 * hh : 64 * hh + 64] = lin2_w[:, jlo : jlo + JH].T
        for t in range(n_kt):
            k0 = 128 * t
            frows = max(0, min(128, MODES - k0))
            if frows > 0:
                wbm[0:frows, L2C + JSH * t : L2C + JSH * t + JSH] = atT[
                    k0 : k0 + frows, j0 : j0 + JSH
                ]
                sbm[0:frows, BSH * t : BSH * t + BSH] = s2qT[
                    k0 : k0 + frows, b0 : b0 + BSH
                ]
            for r in range(frows, 128):
                k = k0 + r
                if k == MODES:                        # bias row
                    wbm[r, L2C + JSH * t : L2C + JSH * t + JSH] = lin1_b[j0 : j0 + JSH]
                    sbm[r, BSH * t : BSH * t + BSH] = np_fp8(1.0)
                elif MODES < k < MODES + 1 + len(ext[bg]):
                    be = ext[bg][k - MODES - 1]
                    wbm[r, L2C + JSH * t : L2C + JSH * t + JSH] = vrows[be][
                        j0 : j0 + JSH
                    ]
                    sbm[r, BSH * t + (be - b0)] = np_fp8(1.0)
        in_maps.append(
            {
                "wb": np.ascontiguousarray(wbm.astype(bf16)),
                "sb": np.ascontiguousarray(sbm),
            }
        )
    return in_maps, n_kt


def kernel(
    params,
    wfft0_re,
    wfft0_im,
    wfft1_re,
    wfft1_im,
    wfft2_re,
    wfft2_im,
    lin1_w,
    lin1_b,
    lin2_w,
    lin2_b,
):
    from concourse.bass_utils import run_bass_kernel_spmd

    in_maps, n_kt = _host_prep(
        params, wfft0_re, wfft0_im, wfft1_re, wfft1_im, wfft2_re, wfft2_im,
        lin1_w, lin1_b, lin2_w,
    )

    nc = _build_program(n_kt)
    _CACHE["last_run"] = (nc, in_maps)
    res = run_bass_kernel_spmd(nc, in_maps, list(range(NCORES)))

    # host: sum j-shard partials per b-shard, add lin2 bias, sigmoid
    pre = np.zeros((L, B), np.float64)
    for c in range(NCORES):
        bg = c % BG
        pre[:, BSH * bg : BSH * (bg + 1)] += res.results[c]["outp"].astype(np.float64)
    out = 1.0 / (1.0 + np.exp(-(pre.T + lin2_b.astype(np.float64))))
    return out.astype(np.float32)
